# revision 11
# baseline (speedup 1.0000x reference)
"""nn_MaxDistance Trainium2 kernel.

Problem: x, y: [8, 4096, 3] f32. Per batch b:
  d2[n,m] = ||x[b,n] - y[b,m]||^2
  h2[b] = max( max_n min_m d2, max_m min_n d2 )
  output = mean_b sqrt(h2[b])   (scalar f32)

Sharding: batch b -> NeuronCore b (8 cores, data parallel). Each core
computes its full 4096x4096 distance/min/max reduction; the final mean over
the 8 per-batch scalars is done on host (tiny all-reduce).

Device algorithm (per core):
  - The pairwise squared distance is computed on the TensorEngine as an
    augmented inner product: with a~ = (x0,x1,x2,||x||^2,1) and
    b~ = (-2y0,-2y1,-2y2,1,||y||^2),  d2[n,m] = a~_n . b~_m.
  - For full PE speed with near-fp32 accuracy, each f32 input value v is
    split on host into bf16 hi/lo parts (v = vh + vl); the K=5 augmented
    product becomes a K=15 bf16 matmul computing ah.bh + al.bh + ah.bl
    (the al.bl term, ~2^-18 relative, is dropped).
  - Each a-tile of 128 points is matmul'd against all 4096 b-points in
    512-column chunks into PSUM (f32), and the VectorEngine min-reduces
    PSUM groups into per-point minima; then max across points via
    reduce_max + a gpsimd partition_all_reduce, and the two directions are
    combined with an elementwise max. A single [1,1] f32 (squared
    Hausdorff) is DMA'd out per core.
"""

import numpy as np
import ml_dtypes

import concourse.bacc as bacc
import concourse.tile as tile
from concourse import mybir
from concourse import bass_utils
from concourse import bass_isa

P = 128
NPTS = 4096
D = 3
K = 15  # 5 augmented dims x 3 bf16 hi/lo product terms
BCH = 512  # matmul moving free dim (one PSUM bank of f32)
BIG = float(np.finfo(np.float32).max) / 4

BF16 = ml_dtypes.bfloat16

# variant: "reduce" = plain PSUM reduce_min (DVE only)
#          "mix16"  = ScalarE converts 6 of 8 PSUM banks per a-tile to fp16
#                     in SBUF; DVE min-combines those at 2x rate and
#                     reduces the remaining 2 banks directly in fp32
#          "ttr"    = tensor_tensor_reduce pairing (crashes TRN2 runtime —
#                     min-reduce uop missing; kept for reference)
VARIANT = "mix16"
MIX16_SPLIT_CP = True
GROUP = 2048  # b-columns consumed per DVE reduce op group

_NC_CACHE = {}


def _build_nc(variant=VARIANT, group=GROUP, npts=NPTS):
    if variant == "mix16":
        group = npts  # whole a-tile row in PSUM; bank-level deps pipeline it
    ntiles = npts // P
    ngroups = npts // group
    nmm = group // BCH
    half = group // 2
    psum_bufs = 1 if variant == "mix16" else 2

    nc = bacc.Bacc("TRN2", target_bir_lowering=False, debug=False)
    dt = mybir.dt

    ins = {}
    for name in ("xa", "yb", "ya", "xb"):
        ins[name] = nc.dram_tensor(name, [K, npts], dt.bfloat16,
                                   kind="ExternalInput").ap()
    out = nc.dram_tensor("h2", [1, 1], dt.float32, kind="ExternalOutput").ap()

    with tile.TileContext(nc) as tc:
        with (
            tc.tile_pool(name="singles", bufs=1) as singles,
            tc.tile_pool(name="psum", bufs=psum_bufs, space="PSUM") as psum_pool,
            tc.tile_pool(name="cp", bufs=3) as cp_pool,
            tc.tile_pool(name="trash", bufs=1) as trash_pool,
            tc.tile_pool(name="accs", bufs=1) as accs_pool,
            tc.tile_pool(name="fin", bufs=1) as fin_pool,
        ):
            ab = {}
            for name in ("xa", "yb", "ya", "xb"):
                t = singles.tile([K, npts], dt.bfloat16, tag=name,
                                 name=f"pts_{name}")
                nc.sync.dma_start(out=t, in_=ins[name])
                ab[name] = t

            dirs = ((ab["xa"], ab["yb"]), (ab["ya"], ab["xb"]))
            accs = [accs_pool.tile([P, ntiles, ngroups], dt.float32,
                                   name=f"acc{d}") for d in range(2)]
            if variant == "ttr":
                dummy = trash_pool.tile([P, 1], dt.float32, name="dummy")

            for d, (A, B) in enumerate(dirs):
                for t in range(ntiles):
                    lhsT = A[:, t * P:(t + 1) * P]
                    for g in range(ngroups):
                        pp = psum_pool.tile([P, group], dt.float32, tag="pp")
                        for j in range(nmm):
                            nc.tensor.matmul(
                                out=pp[:, j * BCH:(j + 1) * BCH],
                                lhsT=lhsT,
                                rhs=B[:, g * group + j * BCH:
                                      g * group + (j + 1) * BCH],
                                start=True, stop=True,
                            )
                        if variant == "mix16":
                            # banks 0-5 -> fp16 SBUF via ScalarE, banks 6-7
                            # reduced directly from PSUM in fp32 on the DVE.
                            ncp = 3 if MIX16_SPLIT_CP else 1
                            cps = []
                            for ci in range(ncp):
                                w = 3072 // ncp
                                cp = cp_pool.tile([P, w], dt.float16,
                                                  tag=f"cp{ci}")
                                nc.scalar.copy(
                                    out=cp, in_=pp[:, ci * w:(ci + 1) * w])
                                cps.append(cp)
                            if ncp == 1:
                                cps = [cps[0][:, 0:1024], cps[0][:, 1024:2048],
                                       cps[0][:, 2048:3072]]
                            r67 = trash_pool.tile([P, 1], dt.float32,
                                                  tag="r67", bufs=2)
                            nc.vector.tensor_reduce(
                                out=r67, in_=pp[:, 3072:4096],
                                axis=mybir.AxisListType.X,
                                op=mybir.AluOpType.min)
                            t1 = cp_pool.tile([P, 1024], dt.float16, tag="t1")
                            nc.vector.tensor_tensor(
                                out=t1, in0=cps[0], in1=cps[1],
                                op=mybir.AluOpType.min)
                            t2 = cp_pool.tile([P, 1024], dt.float16, tag="t2")
                            nc.vector.tensor_tensor(
                                out=t2, in0=t1, in1=cps[2],
                                op=mybir.AluOpType.min)
                            t3 = cp_pool.tile([P, 512], dt.float16, tag="t3")
                            nc.vector.tensor_tensor(
                                out=t3, in0=t2[:, 0:512], in1=t2[:, 512:1024],
                                op=mybir.AluOpType.min)
                            t4 = trash_pool.tile([P, 1], dt.float16,
                                                 tag="t4", bufs=2)
                            nc.vector.tensor_reduce(
                                out=t4, in_=t3, axis=mybir.AxisListType.X,
                                op=mybir.AluOpType.min)
                            nc.vector.tensor_tensor(
                                out=accs[d][:, t, g:g + 1], in0=r67, in1=t4,
                                op=mybir.AluOpType.min)
                        elif variant == "ttr":
                            cp = cp_pool.tile([P, half], dt.float32, tag="cp")
                            nc.scalar.copy(out=cp, in_=pp[:, half:group])
                            nc.vector.tensor_tensor_reduce(
                                out=dummy.broadcast_to((P, half)),
                                in0=pp[:, 0:half],
                                in1=cp,
                                scale=1.0,
                                scalar=BIG,
                                op0=mybir.AluOpType.min,
                                op1=mybir.AluOpType.min,
                                accum_out=accs[d][:, t, g:g + 1],
                            )
                        else:
                            nc.vector.tensor_reduce(
                                out=accs[d][:, t, g:g + 1], in_=pp,
                                axis=mybir.AxisListType.X,
                                op=mybir.AluOpType.min)

            hmaxes = fin_pool.tile([P, 2], dt.float32, name="hmaxes")
            for d in range(2):
                amin = fin_pool.tile([P, ntiles], dt.float32, name=f"amin{d}")
                nc.vector.tensor_reduce(
                    out=amin, in_=accs[d], axis=mybir.AxisListType.X,
                    op=mybir.AluOpType.min)
                nc.vector.tensor_reduce(
                    out=hmaxes[:, d:d + 1], in_=amin,
                    axis=mybir.AxisListType.X, op=mybir.AluOpType.max)
            hb = fin_pool.tile([P, 1], dt.float32, name="hb")
            nc.vector.tensor_tensor(
                out=hb, in0=hmaxes[:, 0:1], in1=hmaxes[:, 1:2],
                op=mybir.AluOpType.max)
            hred = fin_pool.tile([P, 1], dt.float32, name="hred")
            nc.gpsimd.partition_all_reduce(
                out_ap=hred, in_ap=hb, channels=P,
                reduce_op=bass_isa.ReduceOp.max)
            nc.sync.dma_start(out=out, in_=hred[0:1, 0:1])

    nc.compile()
    return nc


def get_nc(**kw):
    key = tuple(sorted(kw.items()))
    if key not in _NC_CACHE:
        _NC_CACHE[key] = _build_nc(**kw)
    return _NC_CACHE[key]


def _split_rows(rows_f32):
    """rows_f32: [5, n] f32 -> hi/lo interleaved [15, n] bf16 pair pattern.

    For a-side array SA and b-side array SB the matmul computes
    sum_k SA[k].SB[k]; rows are laid out so that per augmented dim i:
      a rows: (ah, al, ah)   b rows: (bh, bh, bl)
    giving ah.bh + al.bh + ah.bl per dim."""
    hi = rows_f32.astype(BF16)
    lo = (rows_f32 - hi.astype(np.float32)).astype(BF16)
    return hi, lo


def _make_core_inputs(xb_, yb_):
    """xb_, yb_: [4096, 3] f32 for one batch -> input dict for one core."""
    def aug_a(p):
        n = (p * p).sum(axis=1, dtype=np.float32)
        return np.stack([p[:, 0], p[:, 1], p[:, 2], n,
                         np.ones_like(n)], 0).astype(np.float32)

    def aug_b(p):
        n = (p * p).sum(axis=1, dtype=np.float32)
        return np.stack([-2 * p[:, 0], -2 * p[:, 1], -2 * p[:, 2],
                         np.ones_like(n), n], 0).astype(np.float32)

    def a_side(rows):
        hi, lo = _split_rows(rows)
        outr = np.empty((K, rows.shape[1]), BF16)
        outr[0::3] = hi
        outr[1::3] = lo
        outr[2::3] = hi
        return outr

    def b_side(rows):
        hi, lo = _split_rows(rows)
        outr = np.empty((K, rows.shape[1]), BF16)
        outr[0::3] = hi
        outr[1::3] = hi
        outr[2::3] = lo
        return outr

    return {
        "xa": np.ascontiguousarray(a_side(aug_a(xb_))),
        "yb": np.ascontiguousarray(b_side(aug_b(yb_))),
        "ya": np.ascontiguousarray(a_side(aug_a(yb_))),
        "xb": np.ascontiguousarray(b_side(aug_b(xb_))),
    }


def kernel(x, y):
    x = np.asarray(x, dtype=np.float32)
    y = np.asarray(y, dtype=np.float32)
    nbatch = x.shape[0]
    nc = get_nc()
    in_maps = [_make_core_inputs(x[b], y[b]) for b in range(nbatch)]
    res = bass_utils.run_bass_kernel_spmd(
        nc, in_maps, core_ids=list(range(nbatch)))
    h2 = np.array([res.results[b]["h2"][0, 0] for b in range(nbatch)],
                  dtype=np.float32)
    return np.float32(np.sqrt(np.maximum(h2, 0.0)).mean())


# revision 15
# speedup vs baseline: 1.0352x; 1.0352x over previous
"""nn_MaxDistance Trainium2 kernel.

Problem: x, y: [8, 4096, 3] f32. Per batch b:
  d2[n,m] = ||x[b,n] - y[b,m]||^2
  h2[b] = max( max_n min_m d2, max_m min_n d2 )
  output = mean_b sqrt(h2[b])   (scalar f32)

Sharding: batch b -> NeuronCore b (8 cores, data parallel). Each core
computes its full 4096x4096 distance/min/max reduction; the final mean over
the 8 per-batch scalars is done on host (tiny all-reduce).

Device algorithm (per core):
  - The pairwise squared distance is computed on the TensorEngine as an
    augmented inner product: with a~ = (x0,x1,x2,||x||^2,1) and
    b~ = (-2y0,-2y1,-2y2,1,||y||^2),  d2[n,m] = a~_n . b~_m.
  - For full PE speed with near-fp32 accuracy, each f32 input value v is
    split on host into bf16 hi/lo parts (v = vh + vl); the K=5 augmented
    product becomes a K=15 bf16 matmul computing ah.bh + al.bh + ah.bl
    (the al.bl term, ~2^-18 relative, is dropped).
  - Each a-tile of 128 points is matmul'd against all 4096 b-points in
    512-column chunks into PSUM (f32), and the VectorEngine min-reduces
    PSUM groups into per-point minima; then max across points via
    reduce_max + a gpsimd partition_all_reduce, and the two directions are
    combined with an elementwise max. A single [1,1] f32 (squared
    Hausdorff) is DMA'd out per core.
"""

import numpy as np
import ml_dtypes

import concourse.bacc as bacc
import concourse.tile as tile
from concourse import mybir
from concourse import bass_utils
from concourse import bass_isa

P = 128
NPTS = 4096
D = 3
K = 15  # 5 augmented dims x 3 bf16 hi/lo product terms
BCH = 512  # matmul moving free dim (one PSUM bank of f32)
BIG = float(np.finfo(np.float32).max) / 4

BF16 = ml_dtypes.bfloat16

# variant: "reduce" = plain PSUM reduce_min (DVE only)
#          "mix16"  = ScalarE converts 6 of 8 PSUM banks per a-tile to fp16
#                     in SBUF; DVE min-combines those at 2x rate and
#                     reduces the remaining 2 banks directly in fp32
#          "ttr"    = tensor_tensor_reduce pairing (crashes TRN2 runtime —
#                     min-reduce uop missing; kept for reference)
VARIANT = "mix16"
MIX16_ACT_BANKS = 5  # of 8 PSUM banks routed through ScalarE
MIX16_NCP = 2       # ScalarE copies per a-tile
MIX16_GPSIMD_T1 = False  # run the first fp16 TT-min fold on GpSimd
GROUP = 2048  # b-columns consumed per DVE reduce op group

_NC_CACHE = {}


def _build_nc(variant=VARIANT, group=GROUP, npts=NPTS):
    if variant == "mix16":
        group = npts  # whole a-tile row in PSUM; bank-level deps pipeline it
    ntiles = npts // P
    ngroups = npts // group
    nmm = group // BCH
    half = group // 2
    psum_bufs = 1 if variant == "mix16" else 2

    nc = bacc.Bacc("TRN2", target_bir_lowering=False, debug=False)
    dt = mybir.dt

    ins = {}
    for name in ("xa", "yb", "ya", "xb"):
        ins[name] = nc.dram_tensor(name, [K, npts], dt.bfloat16,
                                   kind="ExternalInput").ap()
    out = nc.dram_tensor("h2", [1, 1], dt.float32, kind="ExternalOutput").ap()

    with tile.TileContext(nc) as tc:
        with (
            tc.tile_pool(name="singles", bufs=1) as singles,
            tc.tile_pool(name="psum", bufs=psum_bufs, space="PSUM") as psum_pool,
            tc.tile_pool(name="cp", bufs=3) as cp_pool,
            tc.tile_pool(name="trash", bufs=1) as trash_pool,
            tc.tile_pool(name="accs", bufs=1) as accs_pool,
            tc.tile_pool(name="fin", bufs=1) as fin_pool,
        ):
            ab = {}
            for name in ("xa", "yb", "ya", "xb"):
                t = singles.tile([K, npts], dt.bfloat16, tag=name,
                                 name=f"pts_{name}")
                nc.sync.dma_start(out=t, in_=ins[name])
                ab[name] = t

            dirs = ((ab["xa"], ab["yb"]), (ab["ya"], ab["xb"]))
            accs = [accs_pool.tile([P, ntiles, ngroups], dt.float32,
                                   name=f"acc{d}") for d in range(2)]
            if variant == "ttr":
                dummy = trash_pool.tile([P, 1], dt.float32, name="dummy")

            for d, (A, B) in enumerate(dirs):
                for t in range(ntiles):
                    lhsT = A[:, t * P:(t + 1) * P]
                    for g in range(ngroups):
                        pp = psum_pool.tile([P, group], dt.float32, tag="pp")
                        for j in range(nmm):
                            nc.tensor.matmul(
                                out=pp[:, j * BCH:(j + 1) * BCH],
                                lhsT=lhsT,
                                rhs=B[:, g * group + j * BCH:
                                      g * group + (j + 1) * BCH],
                                start=True, stop=True,
                            )
                        if variant == "mix16":
                            # First MIX16_ACT_BANKS banks -> fp16 SBUF via
                            # ScalarE (few wide copies amortize the ACT
                            # per-op init, which dominates at 1024 wide);
                            # remaining banks reduced directly from PSUM in
                            # fp32 on the DVE.
                            acols = MIX16_ACT_BANKS * BCH
                            ncp = MIX16_NCP
                            w = acols // ncp
                            cps = []
                            for ci in range(ncp):
                                cp = cp_pool.tile([P, w], dt.float16,
                                                  tag=f"cp{ci}")
                                nc.scalar.copy(
                                    out=cp, in_=pp[:, ci * w:(ci + 1) * w])
                                cps.append(cp)
                            r67 = trash_pool.tile([P, 1], dt.float32,
                                                  tag="r67", bufs=2)
                            nc.vector.tensor_reduce(
                                out=r67, in_=pp[:, acols:group],
                                axis=mybir.AxisListType.X,
                                op=mybir.AluOpType.min)
                            # fold the fp16 copies with 2x-rate TT-mins,
                            # halving until narrow enough to reduce
                            cur = cps[0]
                            cw = w
                            ti = 0
                            for ci in range(1, ncp):
                                nxt = cp_pool.tile([P, cw], dt.float16,
                                                   tag=f"z{ti}")
                                eng = (nc.gpsimd if MIX16_GPSIMD_T1
                                       else nc.vector)
                                eng.tensor_tensor(
                                    out=nxt, in0=cur, in1=cps[ci],
                                    op=mybir.AluOpType.min)
                                cur = nxt
                                ti += 1
                            while cw > 512:
                                cw //= 2
                                nxt = cp_pool.tile([P, cw], dt.float16,
                                                   tag=f"z{ti}")
                                nc.vector.tensor_tensor(
                                    out=nxt, in0=cur[:, 0:cw],
                                    in1=cur[:, cw:2 * cw],
                                    op=mybir.AluOpType.min)
                                cur = nxt
                                ti += 1
                            t4 = trash_pool.tile([P, 1], dt.float16,
                                                 tag="t4", bufs=2)
                            nc.vector.tensor_reduce(
                                out=t4, in_=cur, axis=mybir.AxisListType.X,
                                op=mybir.AluOpType.min)
                            nc.vector.tensor_tensor(
                                out=accs[d][:, t, g:g + 1], in0=r67, in1=t4,
                                op=mybir.AluOpType.min)
                        elif variant == "ttr":
                            cp = cp_pool.tile([P, half], dt.float32, tag="cp")
                            nc.scalar.copy(out=cp, in_=pp[:, half:group])
                            nc.vector.tensor_tensor_reduce(
                                out=dummy.broadcast_to((P, half)),
                                in0=pp[:, 0:half],
                                in1=cp,
                                scale=1.0,
                                scalar=BIG,
                                op0=mybir.AluOpType.min,
                                op1=mybir.AluOpType.min,
                                accum_out=accs[d][:, t, g:g + 1],
                            )
                        else:
                            nc.vector.tensor_reduce(
                                out=accs[d][:, t, g:g + 1], in_=pp,
                                axis=mybir.AxisListType.X,
                                op=mybir.AluOpType.min)

            hmaxes = fin_pool.tile([P, 2], dt.float32, name="hmaxes")
            for d in range(2):
                amin = fin_pool.tile([P, ntiles], dt.float32, name=f"amin{d}")
                nc.vector.tensor_reduce(
                    out=amin, in_=accs[d], axis=mybir.AxisListType.X,
                    op=mybir.AluOpType.min)
                nc.vector.tensor_reduce(
                    out=hmaxes[:, d:d + 1], in_=amin,
                    axis=mybir.AxisListType.X, op=mybir.AluOpType.max)
            hb = fin_pool.tile([P, 1], dt.float32, name="hb")
            nc.vector.tensor_tensor(
                out=hb, in0=hmaxes[:, 0:1], in1=hmaxes[:, 1:2],
                op=mybir.AluOpType.max)
            hred = fin_pool.tile([P, 1], dt.float32, name="hred")
            nc.gpsimd.partition_all_reduce(
                out_ap=hred, in_ap=hb, channels=P,
                reduce_op=bass_isa.ReduceOp.max)
            nc.sync.dma_start(out=out, in_=hred[0:1, 0:1])

    nc.compile()
    return nc


def get_nc(**kw):
    key = tuple(sorted(kw.items()))
    if key not in _NC_CACHE:
        _NC_CACHE[key] = _build_nc(**kw)
    return _NC_CACHE[key]


def _split_rows(rows_f32):
    """rows_f32: [5, n] f32 -> hi/lo interleaved [15, n] bf16 pair pattern.

    For a-side array SA and b-side array SB the matmul computes
    sum_k SA[k].SB[k]; rows are laid out so that per augmented dim i:
      a rows: (ah, al, ah)   b rows: (bh, bh, bl)
    giving ah.bh + al.bh + ah.bl per dim."""
    hi = rows_f32.astype(BF16)
    lo = (rows_f32 - hi.astype(np.float32)).astype(BF16)
    return hi, lo


def _make_core_inputs(xb_, yb_):
    """xb_, yb_: [4096, 3] f32 for one batch -> input dict for one core."""
    def aug_a(p):
        n = (p * p).sum(axis=1, dtype=np.float32)
        return np.stack([p[:, 0], p[:, 1], p[:, 2], n,
                         np.ones_like(n)], 0).astype(np.float32)

    def aug_b(p):
        n = (p * p).sum(axis=1, dtype=np.float32)
        return np.stack([-2 * p[:, 0], -2 * p[:, 1], -2 * p[:, 2],
                         np.ones_like(n), n], 0).astype(np.float32)

    def a_side(rows):
        hi, lo = _split_rows(rows)
        outr = np.empty((K, rows.shape[1]), BF16)
        outr[0::3] = hi
        outr[1::3] = lo
        outr[2::3] = hi
        return outr

    def b_side(rows):
        hi, lo = _split_rows(rows)
        outr = np.empty((K, rows.shape[1]), BF16)
        outr[0::3] = hi
        outr[1::3] = hi
        outr[2::3] = lo
        return outr

    return {
        "xa": np.ascontiguousarray(a_side(aug_a(xb_))),
        "yb": np.ascontiguousarray(b_side(aug_b(yb_))),
        "ya": np.ascontiguousarray(a_side(aug_a(yb_))),
        "xb": np.ascontiguousarray(b_side(aug_b(xb_))),
    }


def kernel(x, y):
    x = np.asarray(x, dtype=np.float32)
    y = np.asarray(y, dtype=np.float32)
    nbatch = x.shape[0]
    nc = get_nc()
    in_maps = [_make_core_inputs(x[b], y[b]) for b in range(nbatch)]
    res = bass_utils.run_bass_kernel_spmd(
        nc, in_maps, core_ids=list(range(nbatch)))
    h2 = np.array([res.results[b]["h2"][0, 0] for b in range(nbatch)],
                  dtype=np.float32)
    return np.float32(np.sqrt(np.maximum(h2, 0.0)).mean())
